# revision 41
# baseline (speedup 1.0000x reference)
"""Trainium2 Bass kernel: causal multi-head attention with LoRA (B=2, T=2048,
C=1024, 16 heads, r=16), SPMD across 8 NeuronCores.

Sharding: core = (batch, head-group-of-4). QKV + attention are fully local per
core (weights pre-sliced per head group on host); the output projection is
computed as a partial sum over each core's 256 y-features and reduced on host.

v3 design:
- fp8(e4m3) DoubleRow matmuls for the C=1024-contraction phases (u, QKV) and
  the ci-contraction projection, and for the non-diagonal AV accumulation:
  one DR matmul contracts 256 rows in ~0.56x the time of one bf16 matmul.
  Weights are pre-scaled by 64 on the host so w~0.02 values clear the e4m3
  subnormal range; the 1/64 (or 1/4096) is folded into the PSUM->SBUF copy
  scale. Scores stay bf16 (K=64 layout doesn't pair), diagonal AV pairs stay
  bf16 (the causal mask multiply runs on bf16 tiles).
- Phase 4 (attention) is software-pipelined (scores one k-tile-pair ahead of
  AV) so the PE never waits on ScalarE's exp -> HAM clock gate stays at 8/8
  (2.4 GHz). exp is batched [128,1024] per ACTIVATE to amortize the 352-cycle
  ScalarE overhead.
- All biases fold into matmuls via ones-rows (rank 16 -> 17 contraction).
- Normalize: D row PE-broadcast + DVE reciprocal_approx_fast + one multiply.
- v for key tiles 8..15 is deferred into attention blocks 0/1 as PE filler;
  the j-1 projection interleaves into block j. A 3-closure reserve feeds the
  PE through the end-of-kernel flush chain.
- Input DMAs are combined (host pre-shuffles weight layouts) because the Sync
  engine issues DMA descriptors serially at ~0.65us each.
"""
import os
import sys

sys.path.insert(0, "/opt/trn_rl_repo")

import numpy as np

import concourse.bass as bass  # noqa: F401
import concourse.bacc as bacc
import concourse.tile as tile
import concourse.mybir as mybir
from concourse.bass_utils import run_bass_kernel_spmd

B, T, C = 2, 2048, 1024
H, HD = 16, 64
R = 16
LORA_SCALE = 1.0 / R
N_CORES = 8
GPB = N_CORES // B          # core groups per batch = 4
HPC = H // GPB              # heads per core = 4
CI = HPC * HD               # per-core y features = 256
P = 128
T5 = T // 512               # 4  (512-wide query blocks)
T1 = T // P                 # 16 (128-wide key tiles)
CT = C // P                 # 8  (128-wide c tiles)
FQK = 2 * HPC * HD // P     # 4  (128-wide qk feature tiles: f0,f1=q f2,f3=k)
F32 = mybir.dt.float32
BF16 = mybir.dt.bfloat16
FP8 = mybir.dt.float8e4
DR = mybir.MatmulPerfMode.DoubleRow

# fp8 measured: rel_err 5.7e-2 (fails 2e-2) -- for zero-mean dot products the
# per-element quantization error does NOT average out (signal and error both
# grow as sqrt(n)), so every fp8 stage costs its full ~4-6% element error.
FP8_QKV = False
FP8_AV = False
FP8_PROJ = False

WS = 64.0 if FP8_QKV else 1.0        # host prescale on x-side weights
PS = 64.0 if FP8_PROJ else 1.0       # host prescale on proj weights
VD = 68 if FP8_AV else 65            # v tile cols (pad to 16B pair stride)

LAST_RESULTS = None
_CACHE = {}


def build():
    nc = bacc.Bacc("TRN2", target_bir_lowering=False, debug=False,
                   num_devices=N_CORES)

    xdt = FP8 if FP8_QKV else BF16
    pdt = FP8 if FP8_PROJ else BF16
    vdt = FP8 if FP8_AV else BF16

    xt_d = nc.dram_tensor("xt", [C, T], xdt, kind="ExternalInput").ap()
    wqk_d = nc.dram_tensor("wqk", [P, CT * FQK * P], xdt, kind="ExternalInput").ap()
    wv_d = nc.dram_tensor("wv", [P, CT * CI], BF16, kind="ExternalInput").ap()
    laa_d = nc.dram_tensor("laa", [P, CT * R], xdt, kind="ExternalInput").ap()
    lbaqk_d = nc.dram_tensor("lbaqk", [R + 1, 2 * CI], BF16, kind="ExternalInput").ap()
    lbav_d = nc.dram_tensor("lbav", [R + 1, CI], BF16, kind="ExternalInput").ap()
    wp_d = nc.dram_tensor("wp", [P, 2 * CT * P], pdt, kind="ExternalInput").ap()
    lap_d = nc.dram_tensor("lap", [P, 2 * R], pdt, kind="ExternalInput").ap()
    lbp_d = nc.dram_tensor("lbp", [R + 1, C], BF16, kind="ExternalInput").ap()
    masks_d = nc.dram_tensor("masks", [P, 896], BF16, kind="ExternalInput").ap()
    onesr_d = nc.dram_tensor("onesr", [1, T], BF16, kind="ExternalInput").ap()
    vones_d = nc.dram_tensor("vones", [P, T1 * HPC * (VD - HD)], vdt,
                             kind="ExternalInput").ap()
    out_d = nc.dram_tensor("out", [C, T], BF16, kind="ExternalOutput").ap()

    with tile.TileContext(nc) as tc:
        with (
            tc.tile_pool(name="const", bufs=1) as cp,
            tc.tile_pool(name="work", bufs=2) as wk,
            tc.tile_pool(name="att", bufs=3) as ap_,
            tc.tile_pool(name="ps", bufs=2, space="PSUM") as ps,
            tc.tile_pool(name="pss", bufs=2, space="PSUM") as pss,
            tc.tile_pool(name="psav", bufs=2, space="PSUM") as psav,
        ):
            # ---- resident SBUF tensors -------------------------------------
            xt_sb = cp.tile([P, CT, T], xdt)             # x^T
            wqk_sb = cp.tile([P, CT // 2, 2, FQK, P], xdt)
            wv_sb = cp.tile([P, CT, CI], BF16)           # W_v^T
            laa_sb = cp.tile([P, CT // 2, 2, R], xdt)    # A_attn^T (c-paired)
            lbaqk_sb = cp.tile([R + 1, FQK, P], BF16)    # [B_qk*WS; bqk*WS]
            lbav_sb = cp.tile([R + 1, CI], BF16)         # [B_v/16; bv]
            wp_sb = cp.tile([P, 2, CT, P], pdt)          # W_proj^T slice
            lap_sb = cp.tile([P, 2, R], pdt)             # A_proj^T slice
            lbp_sb = cp.tile([R + 1, CT, P], BF16)       # [B_p*PS; bp*PS]
            qk_sb = cp.tile([P, FQK, T], BF16)           # q,k feat-major
            v_sb = cp.tile([P, T1, HPC, VD], vdt)        # v natural + ones+pad
            u_sb = cp.tile([R + 1, T], BF16)             # lora-u + ones row
            up_sb = cp.tile([R + 1, T], BF16)            # proj-lora u + ones
            yt_sb = cp.tile([P, 2, T], pdt)              # y^T (ci-major)
            masks = cp.tile([P, 896], BF16)              # causal masks
            onesb = cp.tile([1, HD], BF16)               # bcast stationary

            # ---- input DMAs (few, ordered; sync issues ~0.65us each) -------
            for c in range(2):
                nc.sync.dma_start(out=xt_sb[:, c, :], in_=xt_d[c * P:(c + 1) * P, :])
            nc.sync.dma_start(out=wqk_sb[:, 0:2, :, :, :],
                              in_=wqk_d[:, 0:2 * FQK * P * 2])
            nc.sync.dma_start(out=wqk_sb[:, 2:4, :, :, :],
                              in_=wqk_d[:, 2 * FQK * P * 2:])
            nc.sync.dma_start(out=laa_sb[:], in_=laa_d[:])
            nc.sync.dma_start(out=lbaqk_sb[:], in_=lbaqk_d[:])
            nc.sync.dma_start(out=u_sb[R:R + 1, :], in_=onesr_d[:])
            for c in range(2, CT):
                nc.sync.dma_start(out=xt_sb[:, c, :], in_=xt_d[c * P:(c + 1) * P, :])
            nc.sync.dma_start(out=wv_sb[:], in_=wv_d[:])
            nc.sync.dma_start(out=lbav_sb[:], in_=lbav_d[:])
            nc.sync.dma_start(out=masks[:], in_=masks_d[:])
            nc.sync.dma_start(out=v_sb[:, :, :, HD:VD], in_=vones_d[:])
            nc.sync.dma_start(out=up_sb[R:R + 1, :], in_=onesr_d[:])
            nc.sync.dma_start(out=onesb[:], in_=onesr_d[0:1, 0:HD])
            nc.sync.dma_start(out=wp_sb[:], in_=wp_d[:])
            nc.sync.dma_start(out=lap_sb[:], in_=lap_d[:])
            nc.sync.dma_start(out=lbp_sb[:], in_=lbp_d[:])

            # ---- PE pre-warm -----------------------------------------------
            # The HAM clock gate needs ~3.4us of sustained PE activity to
            # raise the PE clock from 1.2 to 2.4 GHz. Burn dummy matmuls on a
            # scratch PSUM bank while the input DMAs stream, so phases 1-2
            # run at full clock from their first real matmul.
            scr = cp.tile([P, 640], BF16)
            nc.vector.memset(scr[:], 0)
            pwarm = psav.tile([VD, 512], F32, tag="psav", name="pwarm")

            def dummy_mm(n):
                for _ in range(n):
                    nc.tensor.matmul(pwarm[:], scr[:, 0:VD],
                                     scr[:, P:640], start=True, stop=True)

            dummy_mm(16)

            # ---- phase 1 (j=0,1) + phase 2 f=0, streamed behind the x DMAs -
            # Both accumulate c-major: each landing x tile feeds 6 matmuls
            # (2 for u j=0/1, 4 for the f=0 qk features), so the PE streams
            # at the DMA arrival rate instead of idling through phase 1 and
            # then waiting for the full x to start phase 2.
            pus = [ps.tile([R, 512], F32, tag="ps", name=f"puA_{d}")
                   for d in range(2)]
            pqf0 = [pss.tile([P, 2, 512], F32, tag="pss", name=f"pqf0_{k}")
                    for k in range(2)]
            for c in range(CT):
                cc, e = c // 2, c % 2
                for d in range(2):
                    nc.tensor.matmul(pus[d][:], laa_sb[:, cc, e, :],
                                     xt_sb[:, c, d * 512:(d + 1) * 512],
                                     start=(c == 0), stop=(c == CT - 1))
                for j in range(T5):
                    nc.tensor.matmul(pqf0[j // 2][:, j % 2, :],
                                     wqk_sb[:, cc, e, 0, :],
                                     xt_sb[:, c, j * 512:(j + 1) * 512],
                                     start=(c == 0), stop=False)
            for d in range(2):
                nc.scalar.mul(u_sb[0:R, d * 512:(d + 1) * 512], pus[d][:],
                              1.0 / WS)
            # phase 1 (j=2,3): x is resident now, runs dense
            pus2 = [ps.tile([R, 512], F32, tag="ps", name=f"puB_{d}")
                    for d in range(2)]
            for c in range(CT):
                cc, e = c // 2, c % 2
                for d in range(2):
                    nc.tensor.matmul(pus2[d][:], laa_sb[:, cc, e, :],
                                     xt_sb[:, c, (2 + d) * 512:(3 + d) * 512],
                                     start=(c == 0), stop=(c == CT - 1))
            for d in range(2):
                nc.scalar.mul(u_sb[0:R, (2 + d) * 512:(3 + d) * 512],
                              pus2[d][:], 1.0 / WS)
            # f=0 lora tails + copies
            for j in range(T5):
                nc.tensor.matmul(pqf0[j // 2][:, j % 2, :], lbaqk_sb[:, 0, :],
                                 u_sb[:, j * 512:(j + 1) * 512],
                                 start=False, stop=True)
                nc.scalar.mul(qk_sb[:, 0, j * 512:(j + 1) * 512],
                              pqf0[j // 2][:, j % 2, :], 1.0 / WS)

            # ---- phase 2: qk^T = W_qk @ x^T + [B_qk;bqk] @ [u;1] -----------
            for f in (2, 1, 3):
                for j in range(T5):
                    pq = ps.tile([P, 512], F32, tag="ps", name=f"pq{f}_{j}")
                    for cc in range(CT // 2):
                        for e in range(2):
                            nc.tensor.matmul(
                                pq[:], wqk_sb[:, cc, e, f, :],
                                xt_sb[:, 2 * cc + e,
                                      j * 512:(j + 1) * 512],
                                start=(cc == 0 and e == 0), stop=False)
                    nc.tensor.matmul(pq[:], lbaqk_sb[:, f, :],
                                     u_sb[:, j * 512:(j + 1) * 512],
                                     start=False, stop=True)
                    nc.scalar.mul(qk_sb[:, f, j * 512:(j + 1) * 512], pq[:],
                                  1.0 / WS)

            # ---- phase 3: V natural = x @ W_v^T + [u;1]^T @ [B_v;bv] -------
            # Key tiles 8..15 are only consumed by attention blocks 2-3, so
            # their v computation is deferred into blocks 0/1 as PE filler.
            _pv = {}

            def emit_v(i, part=None):
                # part=None: whole tile; 0/1/2: injection-sized chunks
                if part in (None, 0):
                    _pv[i] = ps.tile([P, HPC, HD], F32, tag="ps",
                                     name=f"pv{i}")
                pv = _pv[i]
                cr = {None: range(CT), 0: range(3), 1: range(3, 6),
                      2: range(6, CT)}[part]
                for c in cr:
                    nc.tensor.matmul(pv[:], xt_sb[:, c, i * P:(i + 1) * P],
                                     wv_sb[:, c, :],
                                     start=(c == 0), stop=False)
                if part in (None, 2):
                    nc.tensor.matmul(pv[:], u_sb[:, i * P:(i + 1) * P],
                                     lbav_sb[:], start=False, stop=True)
                    nc.scalar.copy(v_sb[:, i, :, 0:HD], pv[:])
                    del _pv[i]

            for i in range(6):
                emit_v(i)

            # ---- phase 4 + interleaved phase 5/6 ---------------------------
            DRAIN = {"on": False}
            flush_queue = []   # units awaiting the normalize chain
            fill_queue = [(lambda i=i, pt=pt: emit_v(i, pt))
                          for i in range(6, T1) for pt in range(3)]
            proj_queue = []    # closures: one PE-group of proj work each

            def emit_flush(fast=False):
                pav, h, j = flush_queue.pop(0)
                # D row (PSUM) -> SBUF, broadcast to 64 partitions, fast
                # reciprocal, then y^T = yu^T * (1/D). The broadcast runs on
                # GPSIMD (off the PE) except for the end-of-kernel flush,
                # where the PE-matmul broadcast chain is ~1.2us shorter and
                # the PE is idle anyway.
                rsb = wk.tile([HD, 512], F32, tag="rsb", name=f"rsb{h}_{j}")
                if fast:
                    bsb = wk.tile([1, 512], BF16, tag="bsbf",
                                  name=f"bsbf{h}_{j}")
                    nc.vector.tensor_scalar_add(bsb[:], pav[HD:HD + 1, :], 0.0)
                    pb = ps.tile([HD, 512], F32, tag="ps", name=f"pb{h}_{j}")
                    nc.tensor.matmul(pb[:], onesb[:], bsb[:],
                                     start=True, stop=True)
                    nc.vector.reciprocal_approx_fast(rsb[:], pb[:])
                else:
                    bsb = wk.tile([1, 512], F32, tag="bsb", name=f"bsb{h}_{j}")
                    nc.vector.tensor_scalar_add(bsb[:], pav[HD:HD + 1, :], 0.0)
                    rrow = wk.tile([1, 512], F32, tag="rrow",
                                   name=f"rr{h}_{j}")
                    nc.vector.reciprocal_approx_fast(rrow[:], bsb[:])
                    nc.gpsimd.partition_broadcast(rsb[:], rrow[:])
                if h % 2 == 0:
                    nc.vector.tensor_tensor(
                        yt_sb[0:HD, h // 2, j * 512:(j + 1) * 512],
                        pav[0:HD, :], rsb[:], mybir.AluOpType.mult)
                else:
                    tsb = wk.tile([HD, 512], pdt, tag="tsb", name=f"tsb{h}_{j}")
                    nc.vector.tensor_tensor(tsb[:], pav[0:HD, :], rsb[:],
                                            mybir.AluOpType.mult)
                    nc.sync.dma_start(
                        out=yt_sb[HD:P, h // 2, j * 512:(j + 1) * 512],
                        in_=tsb[:])

            def make_proj(j):
                def p5():
                    pu = ps.tile([R, 512], F32, tag="ps", name=f"pu5_{j}")
                    if FP8_PROJ:
                        nc.tensor.matmul(pu[:], lap_sb[:],
                                         yt_sb[:, :, j * 512:(j + 1) * 512],
                                         start=True, stop=True, perf_mode=DR)
                    else:
                        for ci in range(2):
                            nc.tensor.matmul(
                                pu[:], lap_sb[:, ci, :],
                                yt_sb[:, ci, j * 512:(j + 1) * 512],
                                start=(ci == 0), stop=(ci == 1))
                    nc.vector.tensor_scalar_mul(
                        up_sb[0:R, j * 512:(j + 1) * 512], pu[:], 1.0 / PS)
                proj_queue.append(p5)
                for co in range(CT):
                    def p6(co=co):
                        po = ps.tile([P, 512], F32, tag="ps",
                                     name=f"po{j}_{co}")
                        if FP8_PROJ:
                            nc.tensor.matmul(
                                po[:], wp_sb[:, :, co, :],
                                yt_sb[:, :, j * 512:(j + 1) * 512],
                                start=True, stop=False, perf_mode=DR)
                        else:
                            for ci in range(2):
                                nc.tensor.matmul(
                                    po[:], wp_sb[:, ci, co, :],
                                    yt_sb[:, ci, j * 512:(j + 1) * 512],
                                    start=(ci == 0), stop=False)
                        nc.tensor.matmul(po[:], lbp_sb[:, co, :],
                                         up_sb[:, j * 512:(j + 1) * 512],
                                         start=False, stop=True)
                        oq = wk.tile([P, 512], BF16, tag="oq",
                                     name=f"oq{j}_{co}")
                        if DRAIN["on"] and co % 2 == 0:
                            nc.scalar.mul(oq[:], po[:], 1.0 / PS)
                        else:
                            nc.vector.tensor_scalar_mul(oq[:], po[:],
                                                        1.0 / PS)
                        nc.sync.dma_start(
                            out=out_d[co * P:(co + 1) * P,
                                      j * 512:(j + 1) * 512],
                            in_=oq[:])
                    proj_queue.append(p6)

            # Flatten all (head, block) units into one stream of k-tile pairs
            # and software-pipeline ACROSS unit boundaries: the score pair for
            # stream item idx is emitted one step ahead of the AV pair for
            # item idx-1, so ScalarE's exp never waits at unit boundaries.
            # Odd heads first per block: their normalize chain ends in an SBUF
            # DMA (partition shift), so the block-final flush is a direct DVE
            # write and the end-of-kernel chain stays short.
            items = []
            for j in range(T5):
                for h in (1, 3, 0, 2):
                    npair = 2 * (j + 1)
                    for p in range(npair):
                        items.append((h, j, p, npair))

            pavs = {}
            at_tiles = {}

            def emit_S(idx):
                h, j, p, npair = items[idx]
                if p == 0:
                    pavs[(h, j)] = psav.tile([VD, 512], F32, tag="psav",
                                             name=f"pav{h}_{j}")
                # Diagonal k-tile pairs align as (a=0,1) or (a=2,3), where
                # a = i - 4j. In diagonal tile a, query columns < 128a are
                # entirely above the causal diagonal: skip them in the score
                # matmul, the exp, and the AV matmul. Only the 128-wide
                # partial strip [128a, 128a+128) needs the mask multiply,
                # and it is the same triangular block masks[:, 384:512].
                f8 = FP8_AV and (2 * p + 1 < 4 * j)
                pst = pss.tile([P, 2, 512], F32, tag="pss",
                               name=f"pst{h}_{j}_{p}")
                fq = h // 2
                pqb = (h % 2) * HD
                for d in (0, 1):
                    i = 2 * p + d
                    a = i - 4 * j
                    lead = 128 * a if a > 0 else 0
                    qt = qk_sb[pqb:pqb + HD, fq,
                               j * 512 + lead:(j + 1) * 512]
                    kt = qk_sb[pqb:pqb + HD, 2 + fq, i * P:(i + 1) * P]
                    nc.tensor.matmul(pst[:, d, lead:512], kt, qt,
                                     start=True, stop=True)
                at = ap_.tile([P, 2, 512], FP8 if f8 else BF16,
                              tag="att8" if f8 else "att16",
                              name=f"at{h}_{j}_{p}")
                skip = 256 if 2 * p - 4 * j >= 2 else 0
                nc.scalar.activation(at[:, :, skip:512],
                                     pst[:, :, skip:512],
                                     mybir.ActivationFunctionType.Exp,
                                     scale=0.125)
                for d in (0, 1):
                    a = 2 * p + d - 4 * j
                    if a >= 0:
                        nc.vector.tensor_tensor(
                            at[:, d, 128 * a:128 * a + 128],
                            at[:, d, 128 * a:128 * a + 128],
                            masks[:, 384:512], mybir.AluOpType.mult)
                at_tiles[idx] = (at, f8)

            def emit_A(idx):
                h, j, p, npair = items[idx]
                at, f8 = at_tiles.pop(idx)
                pav = pavs[(h, j)]
                if f8:
                    nc.tensor.matmul(pav[:], v_sb[:, 2 * p:2 * p + 2, h, :],
                                     at[:], start=(p == 0), stop=False,
                                     perf_mode=DR)
                else:
                    for d in (0, 1):
                        i = 2 * p + d
                        a = i - 4 * j
                        lead = 128 * a if a > 0 else 0
                        nc.tensor.matmul(pav[:, lead:512], v_sb[:, i, h, :],
                                         at[:, d, lead:512],
                                         start=(i == 0),
                                         stop=(i == 2 * npair - 1))
                if p == npair - 1:
                    flush_queue.append((pavs.pop((h, j)), h, j))
                    if h == 2:  # last unit of block j
                        make_proj(j)

            for idx in range(len(items) + 1):
                if idx < len(items):
                    emit_S(idx)
                    h, j, p, npair = items[idx]
                    # flush the previous unit early in this unit: its pb
                    # broadcast fills the PE while this unit's exp completes
                    if p == 1 and flush_queue:
                        emit_flush()
                if idx >= 1:
                    emit_A(idx - 1)
                # inject deferred v eagerly; keep 3 proj closures in reserve
                # to feed the PE through the end-of-kernel flush chain
                if fill_queue:
                    if idx >= 1:
                        fill_queue.pop(0)()
                elif proj_queue and len(proj_queue) > (4 if j == 3 else 3):
                    proj_queue.pop(0)()

            # Drain: reserve closures first (ready now -- keep the PE busy
            # while the final flush chain resolves), final flush in between,
            # dummy matmuls bridging the gaps so the drain stays at 2.4 GHz.
            pwarm = psav.tile([VD, 512], F32, tag="psav", name="pwarm2")
            DRAIN["on"] = True
            while flush_queue or proj_queue:
                for _ in range(3):
                    if proj_queue:
                        proj_queue.pop(0)()
                    dummy_mm(2)
                if flush_queue:
                    emit_flush(fast=True)

    nc.compile()
    return nc


def _shard_inputs(x, w_attn, b_attn, lora_a_attn, lora_b_attn, w_proj, b_proj,
                  lora_a_proj, lora_b_proj):
    import ml_dtypes
    bf16 = ml_dtypes.bfloat16
    fp8 = ml_dtypes.float8_e4m3fn
    xdt = fp8 if FP8_QKV else bf16
    pdt = fp8 if FP8_PROJ else bf16
    vdt = fp8 if FP8_AV else bf16
    f32 = np.float32
    x = np.asarray(x, f32)
    w_attn = np.asarray(w_attn, f32)
    b_attn = np.asarray(b_attn, f32)
    lora_a_attn = np.asarray(lora_a_attn, f32)
    lora_b_attn = np.asarray(lora_b_attn, f32)
    w_proj = np.asarray(w_proj, f32)
    b_proj = np.asarray(b_proj, f32)
    lora_a_proj = np.asarray(lora_a_proj, f32)
    lora_b_proj = np.asarray(lora_b_proj, f32)

    def shuf(a, nchunk):
        # (nchunk*128, F) row-major -> (128, nchunk, F) partition-major
        return np.ascontiguousarray(
            a.reshape(nchunk, P, -1).transpose(1, 0, 2))

    laa2 = shuf(lora_a_attn.T * WS, CT).reshape(P, -1).astype(xdt)
    # masks[p, z] = 1.0 if z >= p + 384 else 0.0
    pp, zz = np.meshgrid(np.arange(P), np.arange(896), indexing="ij")
    masks = (zz >= pp + 384).astype(bf16)
    onesr = np.ones((1, T), bf16)
    vpat = np.zeros((P, T1 * HPC, VD - HD), f32)
    vpat[:, :, 0] = 1.0
    vones = np.ascontiguousarray(vpat.reshape(P, -1)).astype(vdt)
    lbp = np.concatenate(
        [(lora_b_proj * LORA_SCALE).T, np.zeros((1, C), f32)], 0) * PS
    in_maps = []
    for core in range(N_CORES):
        b = core // GPB
        heads = [(core % GPB) * HPC + k for k in range(HPC)]
        q_idx = np.concatenate([np.arange(h * HD, (h + 1) * HD) for h in heads])
        k_idx = q_idx + C
        v_idx = q_idx + 2 * C
        qk_idx = np.concatenate([q_idx, k_idx])
        wqk2 = shuf(w_attn[qk_idx].T * WS, CT).reshape(P, -1).astype(xdt)
        wv2 = shuf(w_attn[v_idx].T, CT).reshape(P, -1).astype(bf16)
        lbaqk = np.concatenate(
            [(lora_b_attn[qk_idx] * LORA_SCALE).T, b_attn[qk_idx][None, :]],
            0) * WS
        lbav = np.concatenate(
            [(lora_b_attn[v_idx] * LORA_SCALE).T, b_attn[v_idx][None, :]], 0)
        wp2 = shuf(w_proj[:, q_idx].T * PS, 2).reshape(P, -1).astype(pdt)
        lap2 = shuf(lora_a_proj[:, q_idx].T * PS, 2).reshape(P, -1).astype(pdt)
        lbp_c = lbp.copy()
        if core % GPB == 0:
            lbp_c[R] = b_proj * PS
        in_maps.append({
            "xt": np.ascontiguousarray(x[b].T).astype(xdt),
            "wqk": wqk2, "wv": wv2,
            "laa": laa2, "lbaqk": np.ascontiguousarray(lbaqk).astype(bf16),
            "lbav": np.ascontiguousarray(lbav).astype(bf16),
            "wp": wp2, "lap": lap2,
            "lbp": np.ascontiguousarray(lbp_c).astype(bf16),
            "masks": masks, "onesr": onesr, "vones": vones,
        })
    return in_maps


def kernel(x, w_attn, b_attn, lora_a_attn, lora_b_attn, w_proj, b_proj,
           lora_a_proj, lora_b_proj, n_head):
    global LAST_RESULTS
    assert int(n_head) == H
    if "nc" not in _CACHE:
        _CACHE["nc"] = build()
    nc = _CACHE["nc"]
    in_maps = _shard_inputs(x, w_attn, b_attn, lora_a_attn, lora_b_attn,
                            w_proj, b_proj, lora_a_proj, lora_b_proj)
    res = run_bass_kernel_spmd(
        nc, in_maps, core_ids=list(range(N_CORES)),
        trace=bool(os.environ.get("BASS_KERNEL_TRACE")))
    LAST_RESULTS = res
    out = np.zeros((B, C, T), np.float32)
    for core in range(N_CORES):
        out[core // GPB] += np.asarray(res.results[core]["out"],
                                       dtype=np.float32)
    return np.ascontiguousarray(out.transpose(0, 2, 1))


# revision 43
# speedup vs baseline: 1.0002x; 1.0002x over previous
"""Trainium2 Bass kernel: causal multi-head attention with LoRA (B=2, T=2048,
C=1024, 16 heads, r=16), SPMD across 8 NeuronCores.

Sharding: core = (batch, head-group-of-4). QKV + attention are fully local per
core (weights pre-sliced per head group on host); the output projection is
computed as a partial sum over each core's 256 y-features and reduced on host.

v3 design:
- fp8(e4m3) DoubleRow matmuls for the C=1024-contraction phases (u, QKV) and
  the ci-contraction projection, and for the non-diagonal AV accumulation:
  one DR matmul contracts 256 rows in ~0.56x the time of one bf16 matmul.
  Weights are pre-scaled by 64 on the host so w~0.02 values clear the e4m3
  subnormal range; the 1/64 (or 1/4096) is folded into the PSUM->SBUF copy
  scale. Scores stay bf16 (K=64 layout doesn't pair), diagonal AV pairs stay
  bf16 (the causal mask multiply runs on bf16 tiles).
- Phase 4 (attention) is software-pipelined (scores one k-tile-pair ahead of
  AV) so the PE never waits on ScalarE's exp -> HAM clock gate stays at 8/8
  (2.4 GHz). exp is batched [128,1024] per ACTIVATE to amortize the 352-cycle
  ScalarE overhead.
- All biases fold into matmuls via ones-rows (rank 16 -> 17 contraction).
- Normalize: D row PE-broadcast + DVE reciprocal_approx_fast + one multiply.
- v for key tiles 8..15 is deferred into attention blocks 0/1 as PE filler;
  the j-1 projection interleaves into block j. A 3-closure reserve feeds the
  PE through the end-of-kernel flush chain.
- Input DMAs are combined (host pre-shuffles weight layouts) because the Sync
  engine issues DMA descriptors serially at ~0.65us each.
"""
import os
import sys

sys.path.insert(0, "/opt/trn_rl_repo")

import numpy as np

import concourse.bass as bass  # noqa: F401
import concourse.bacc as bacc
import concourse.tile as tile
import concourse.mybir as mybir
from concourse.bass_utils import run_bass_kernel_spmd

B, T, C = 2, 2048, 1024
H, HD = 16, 64
R = 16
LORA_SCALE = 1.0 / R
N_CORES = 8
GPB = N_CORES // B          # core groups per batch = 4
HPC = H // GPB              # heads per core = 4
CI = HPC * HD               # per-core y features = 256
P = 128
T5 = T // 512               # 4  (512-wide query blocks)
T1 = T // P                 # 16 (128-wide key tiles)
CT = C // P                 # 8  (128-wide c tiles)
FQK = 2 * HPC * HD // P     # 4  (128-wide qk feature tiles: f0,f1=q f2,f3=k)
F32 = mybir.dt.float32
BF16 = mybir.dt.bfloat16
FP8 = mybir.dt.float8e4
DR = mybir.MatmulPerfMode.DoubleRow

# fp8 measured: rel_err 5.7e-2 (fails 2e-2) -- for zero-mean dot products the
# per-element quantization error does NOT average out (signal and error both
# grow as sqrt(n)), so every fp8 stage costs its full ~4-6% element error.
FP8_QKV = False
FP8_AV = False
FP8_PROJ = False

WS = 64.0 if FP8_QKV else 1.0        # host prescale on x-side weights
PS = 64.0 if FP8_PROJ else 1.0       # host prescale on proj weights
VD = 68 if FP8_AV else 65            # v tile cols (pad to 16B pair stride)

LAST_RESULTS = None
_CACHE = {}


def build():
    nc = bacc.Bacc("TRN2", target_bir_lowering=False, debug=False,
                   num_devices=N_CORES)

    xdt = FP8 if FP8_QKV else BF16
    pdt = FP8 if FP8_PROJ else BF16
    vdt = FP8 if FP8_AV else BF16

    xt_d = nc.dram_tensor("xt", [C, T], xdt, kind="ExternalInput").ap()
    wqk_d = nc.dram_tensor("wqk", [P, CT * FQK * P], xdt, kind="ExternalInput").ap()
    wv_d = nc.dram_tensor("wv", [P, CT * CI], BF16, kind="ExternalInput").ap()
    laa_d = nc.dram_tensor("laa", [P, CT * R], xdt, kind="ExternalInput").ap()
    lbaqk_d = nc.dram_tensor("lbaqk", [R + 1, 2 * CI], BF16, kind="ExternalInput").ap()
    lbav_d = nc.dram_tensor("lbav", [R + 1, CI], BF16, kind="ExternalInput").ap()
    wp_d = nc.dram_tensor("wp", [P, 2 * CT * P], pdt, kind="ExternalInput").ap()
    lap_d = nc.dram_tensor("lap", [P, 2 * R], pdt, kind="ExternalInput").ap()
    lbp_d = nc.dram_tensor("lbp", [R + 1, C], BF16, kind="ExternalInput").ap()
    masks_d = nc.dram_tensor("masks", [P, 896], BF16, kind="ExternalInput").ap()
    onesr_d = nc.dram_tensor("onesr", [1, T], BF16, kind="ExternalInput").ap()
    vones_d = nc.dram_tensor("vones", [P, T1 * HPC * (VD - HD)], vdt,
                             kind="ExternalInput").ap()
    out_d = nc.dram_tensor("out", [C, T], BF16, kind="ExternalOutput").ap()

    with tile.TileContext(nc) as tc:
        with (
            tc.tile_pool(name="const", bufs=1) as cp,
            tc.tile_pool(name="work", bufs=2) as wk,
            tc.tile_pool(name="att", bufs=3) as ap_,
            tc.tile_pool(name="ps", bufs=2, space="PSUM") as ps,
            tc.tile_pool(name="pss", bufs=2, space="PSUM") as pss,
            tc.tile_pool(name="psav", bufs=2, space="PSUM") as psav,
        ):
            # ---- resident SBUF tensors -------------------------------------
            xt_sb = cp.tile([P, CT, T], xdt)             # x^T
            wqk_sb = cp.tile([P, CT // 2, 2, FQK, P], xdt)
            wv_sb = cp.tile([P, CT, CI], BF16)           # W_v^T
            laa_sb = cp.tile([P, CT // 2, 2, R], xdt)    # A_attn^T (c-paired)
            lbaqk_sb = cp.tile([R + 1, FQK, P], BF16)    # [B_qk*WS; bqk*WS]
            lbav_sb = cp.tile([R + 1, CI], BF16)         # [B_v/16; bv]
            wp_sb = cp.tile([P, 2, CT, P], pdt)          # W_proj^T slice
            lap_sb = cp.tile([P, 2, R], pdt)             # A_proj^T slice
            lbp_sb = cp.tile([R + 1, CT, P], BF16)       # [B_p*PS; bp*PS]
            qk_sb = cp.tile([P, FQK, T], BF16)           # q,k feat-major
            v_sb = cp.tile([P, T1, HPC, VD], vdt)        # v natural + ones+pad
            u_sb = cp.tile([R + 1, T], BF16)             # lora-u + ones row
            up_sb = cp.tile([R + 1, T], BF16)            # proj-lora u + ones
            yt_sb = cp.tile([P, 2, T], pdt)              # y^T (ci-major)
            masks = cp.tile([P, 896], BF16)              # causal masks
            onesb = cp.tile([1, HD], BF16)               # bcast stationary

            # ---- input DMAs (few, ordered; sync issues ~0.65us each) -------
            for c in range(2):
                nc.sync.dma_start(out=xt_sb[:, c, :], in_=xt_d[c * P:(c + 1) * P, :])
            nc.sync.dma_start(out=wqk_sb[:, 0:2, :, :, :],
                              in_=wqk_d[:, 0:2 * FQK * P * 2])
            nc.sync.dma_start(out=wqk_sb[:, 2:4, :, :, :],
                              in_=wqk_d[:, 2 * FQK * P * 2:])
            nc.sync.dma_start(out=laa_sb[:], in_=laa_d[:])
            nc.sync.dma_start(out=lbaqk_sb[:], in_=lbaqk_d[:])
            nc.sync.dma_start(out=u_sb[R:R + 1, :], in_=onesr_d[:])
            for c in range(2, CT):
                nc.sync.dma_start(out=xt_sb[:, c, :], in_=xt_d[c * P:(c + 1) * P, :])
            nc.sync.dma_start(out=wv_sb[:], in_=wv_d[:])
            nc.sync.dma_start(out=lbav_sb[:], in_=lbav_d[:])
            nc.sync.dma_start(out=masks[:], in_=masks_d[:])
            nc.sync.dma_start(out=v_sb[:, :, :, HD:VD], in_=vones_d[:])
            nc.sync.dma_start(out=up_sb[R:R + 1, :], in_=onesr_d[:])
            nc.sync.dma_start(out=onesb[:], in_=onesr_d[0:1, 0:HD])
            nc.sync.dma_start(out=wp_sb[:], in_=wp_d[:])
            nc.sync.dma_start(out=lap_sb[:], in_=lap_d[:])
            nc.sync.dma_start(out=lbp_sb[:], in_=lbp_d[:])

            # ---- PE pre-warm -----------------------------------------------
            # The HAM clock gate needs ~3.4us of sustained PE activity to
            # raise the PE clock from 1.2 to 2.4 GHz. Burn dummy matmuls on a
            # scratch PSUM bank while the input DMAs stream, so phases 1-2
            # run at full clock from their first real matmul.
            scr = cp.tile([P, 640], BF16)
            nc.vector.memset(scr[:], 0)
            pwarm = psav.tile([VD, 512], F32, tag="psav", name="pwarm")

            def dummy_mm(n):
                for _ in range(n):
                    nc.tensor.matmul(pwarm[:], scr[:, 0:VD],
                                     scr[:, P:640], start=True, stop=True)

            dummy_mm(16)

            # ---- phase 1 (j=0,1) + phase 2 f=0, streamed behind the x DMAs -
            # Both accumulate c-major: each landing x tile feeds 6 matmuls
            # (2 for u j=0/1, 4 for the f=0 qk features), so the PE streams
            # at the DMA arrival rate instead of idling through phase 1 and
            # then waiting for the full x to start phase 2.
            pus = [ps.tile([R, 512], F32, tag="ps", name=f"puA_{d}")
                   for d in range(2)]
            pqf0 = [pss.tile([P, 2, 512], F32, tag="pss", name=f"pqf0_{k}")
                    for k in range(2)]
            for c in range(CT):
                cc, e = c // 2, c % 2
                for d in range(2):
                    nc.tensor.matmul(pus[d][:], laa_sb[:, cc, e, :],
                                     xt_sb[:, c, d * 512:(d + 1) * 512],
                                     start=(c == 0), stop=(c == CT - 1))
                for j in range(T5):
                    nc.tensor.matmul(pqf0[j // 2][:, j % 2, :],
                                     wqk_sb[:, cc, e, 0, :],
                                     xt_sb[:, c, j * 512:(j + 1) * 512],
                                     start=(c == 0), stop=False)
            for d in range(2):
                nc.scalar.mul(u_sb[0:R, d * 512:(d + 1) * 512], pus[d][:],
                              1.0 / WS)
            # phase 1 (j=2,3): x is resident now, runs dense
            pus2 = [ps.tile([R, 512], F32, tag="ps", name=f"puB_{d}")
                    for d in range(2)]
            for c in range(CT):
                cc, e = c // 2, c % 2
                for d in range(2):
                    nc.tensor.matmul(pus2[d][:], laa_sb[:, cc, e, :],
                                     xt_sb[:, c, (2 + d) * 512:(3 + d) * 512],
                                     start=(c == 0), stop=(c == CT - 1))
            for d in range(2):
                nc.scalar.mul(u_sb[0:R, (2 + d) * 512:(3 + d) * 512],
                              pus2[d][:], 1.0 / WS)
            # f=0 lora tails + copies
            for j in range(T5):
                nc.tensor.matmul(pqf0[j // 2][:, j % 2, :], lbaqk_sb[:, 0, :],
                                 u_sb[:, j * 512:(j + 1) * 512],
                                 start=False, stop=True)
                nc.scalar.mul(qk_sb[:, 0, j * 512:(j + 1) * 512],
                              pqf0[j // 2][:, j % 2, :], 1.0 / WS)

            # ---- phase 2: qk^T = W_qk @ x^T + [B_qk;bqk] @ [u;1] -----------
            for f in (2, 1, 3):
                for j in range(T5):
                    pq = ps.tile([P, 512], F32, tag="ps", name=f"pq{f}_{j}")
                    for cc in range(CT // 2):
                        for e in range(2):
                            nc.tensor.matmul(
                                pq[:], wqk_sb[:, cc, e, f, :],
                                xt_sb[:, 2 * cc + e,
                                      j * 512:(j + 1) * 512],
                                start=(cc == 0 and e == 0), stop=False)
                    nc.tensor.matmul(pq[:], lbaqk_sb[:, f, :],
                                     u_sb[:, j * 512:(j + 1) * 512],
                                     start=False, stop=True)
                    nc.scalar.mul(qk_sb[:, f, j * 512:(j + 1) * 512], pq[:],
                                  1.0 / WS)

            # ---- phase 3: V natural = x @ W_v^T + [u;1]^T @ [B_v;bv] -------
            # Key tiles 8..15 are only consumed by attention blocks 2-3, so
            # their v computation is deferred into blocks 0/1 as PE filler.
            _pv = {}

            def emit_v(i, part=None):
                # part=None: whole tile; 0/1/2: injection-sized chunks
                if part in (None, 0):
                    _pv[i] = ps.tile([P, HPC, HD], F32, tag="ps",
                                     name=f"pv{i}")
                pv = _pv[i]
                cr = {None: range(CT), 0: range(3), 1: range(3, 6),
                      2: range(6, CT)}[part]
                for c in cr:
                    nc.tensor.matmul(pv[:], xt_sb[:, c, i * P:(i + 1) * P],
                                     wv_sb[:, c, :],
                                     start=(c == 0), stop=False)
                if part in (None, 2):
                    nc.tensor.matmul(pv[:], u_sb[:, i * P:(i + 1) * P],
                                     lbav_sb[:], start=False, stop=True)
                    nc.scalar.copy(v_sb[:, i, :, 0:HD], pv[:])
                    del _pv[i]

            for i in range(6):
                emit_v(i)

            # ---- phase 4 + interleaved phase 5/6 ---------------------------
            DRAIN = {"on": False}
            flush_queue = []   # units awaiting the normalize chain
            fill_queue = [(lambda i=i, pt=pt: emit_v(i, pt))
                          for i in range(6, T1) for pt in range(3)]
            proj_queue = []    # closures: one PE-group of proj work each

            def emit_flush(fast=False):
                pav, h, j = flush_queue.pop(0)
                # D row (PSUM) -> SBUF, broadcast to 64 partitions, fast
                # reciprocal, then y^T = yu^T * (1/D). The broadcast runs on
                # GPSIMD (off the PE) except for the end-of-kernel flush,
                # where the PE-matmul broadcast chain is ~1.2us shorter and
                # the PE is idle anyway.
                rsb = wk.tile([HD, 512], F32, tag="rsb", name=f"rsb{h}_{j}")
                if fast:
                    bsb = wk.tile([1, 512], BF16, tag="bsbf",
                                  name=f"bsbf{h}_{j}")
                    nc.vector.tensor_scalar_add(bsb[:], pav[HD:HD + 1, :], 0.0)
                    pb = ps.tile([HD, 512], F32, tag="ps", name=f"pb{h}_{j}")
                    nc.tensor.matmul(pb[:], onesb[:], bsb[:],
                                     start=True, stop=True)
                    nc.vector.reciprocal_approx_fast(rsb[:], pb[:])
                else:
                    bsb = wk.tile([1, 512], F32, tag="bsb", name=f"bsb{h}_{j}")
                    nc.vector.tensor_scalar_add(bsb[:], pav[HD:HD + 1, :], 0.0)
                    rrow = wk.tile([1, 512], F32, tag="rrow",
                                   name=f"rr{h}_{j}")
                    nc.vector.reciprocal_approx_fast(rrow[:], bsb[:])
                    nc.gpsimd.partition_broadcast(rsb[:], rrow[:])
                if h % 2 == 0:
                    nc.vector.tensor_tensor(
                        yt_sb[0:HD, h // 2, j * 512:(j + 1) * 512],
                        pav[0:HD, :], rsb[:], mybir.AluOpType.mult)
                else:
                    tsb = wk.tile([HD, 512], pdt, tag="tsb", name=f"tsb{h}_{j}")
                    nc.vector.tensor_tensor(tsb[:], pav[0:HD, :], rsb[:],
                                            mybir.AluOpType.mult)
                    nc.sync.dma_start(
                        out=yt_sb[HD:P, h // 2, j * 512:(j + 1) * 512],
                        in_=tsb[:])

            def make_proj(j):
                def p5():
                    pu = ps.tile([R, 512], F32, tag="ps", name=f"pu5_{j}")
                    if FP8_PROJ:
                        nc.tensor.matmul(pu[:], lap_sb[:],
                                         yt_sb[:, :, j * 512:(j + 1) * 512],
                                         start=True, stop=True, perf_mode=DR)
                    else:
                        for ci in range(2):
                            nc.tensor.matmul(
                                pu[:], lap_sb[:, ci, :],
                                yt_sb[:, ci, j * 512:(j + 1) * 512],
                                start=(ci == 0), stop=(ci == 1))
                    nc.vector.tensor_scalar_mul(
                        up_sb[0:R, j * 512:(j + 1) * 512], pu[:], 1.0 / PS)
                proj_queue.append(p5)
                for co in range(CT):
                    def p6(co=co):
                        po = ps.tile([P, 512], F32, tag="ps",
                                     name=f"po{j}_{co}")
                        if FP8_PROJ:
                            nc.tensor.matmul(
                                po[:], wp_sb[:, :, co, :],
                                yt_sb[:, :, j * 512:(j + 1) * 512],
                                start=True, stop=False, perf_mode=DR)
                        else:
                            for ci in range(2):
                                nc.tensor.matmul(
                                    po[:], wp_sb[:, ci, co, :],
                                    yt_sb[:, ci, j * 512:(j + 1) * 512],
                                    start=(ci == 0), stop=False)
                        nc.tensor.matmul(po[:], lbp_sb[:, co, :],
                                         up_sb[:, j * 512:(j + 1) * 512],
                                         start=False, stop=True)
                        oq = wk.tile([P, 512], BF16, tag="oq",
                                     name=f"oq{j}_{co}")
                        nc.vector.tensor_scalar_mul(oq[:], po[:], 1.0 / PS)
                        nc.sync.dma_start(
                            out=out_d[co * P:(co + 1) * P,
                                      j * 512:(j + 1) * 512],
                            in_=oq[:])
                    proj_queue.append(p6)

            # Flatten all (head, block) units into one stream of k-tile pairs
            # and software-pipeline ACROSS unit boundaries: the score pair for
            # stream item idx is emitted one step ahead of the AV pair for
            # item idx-1, so ScalarE's exp never waits at unit boundaries.
            # Odd heads first per block: their normalize chain ends in an SBUF
            # DMA (partition shift), so the block-final flush is a direct DVE
            # write and the end-of-kernel chain stays short.
            items = []
            for j in range(T5):
                for h in (1, 3, 0, 2):
                    npair = 2 * (j + 1)
                    for p in range(npair):
                        items.append((h, j, p, npair))

            pavs = {}
            at_tiles = {}

            def emit_S(idx):
                h, j, p, npair = items[idx]
                if p == 0:
                    pavs[(h, j)] = psav.tile([VD, 512], F32, tag="psav",
                                             name=f"pav{h}_{j}")
                # Diagonal k-tile pairs align as (a=0,1) or (a=2,3), where
                # a = i - 4j. In diagonal tile a, query columns < 128a are
                # entirely above the causal diagonal: skip them in the score
                # matmul, the exp, and the AV matmul. Only the 128-wide
                # partial strip [128a, 128a+128) needs the mask multiply,
                # and it is the same triangular block masks[:, 384:512].
                f8 = FP8_AV and (2 * p + 1 < 4 * j)
                pst = pss.tile([P, 2, 512], F32, tag="pss",
                               name=f"pst{h}_{j}_{p}")
                fq = h // 2
                pqb = (h % 2) * HD
                for d in (0, 1):
                    i = 2 * p + d
                    a = i - 4 * j
                    lead = 128 * a if a > 0 else 0
                    qt = qk_sb[pqb:pqb + HD, fq,
                               j * 512 + lead:(j + 1) * 512]
                    kt = qk_sb[pqb:pqb + HD, 2 + fq, i * P:(i + 1) * P]
                    nc.tensor.matmul(pst[:, d, lead:512], kt, qt,
                                     start=True, stop=True)
                at = ap_.tile([P, 2, 512], FP8 if f8 else BF16,
                              tag="att8" if f8 else "att16",
                              name=f"at{h}_{j}_{p}")
                skip = 256 if 2 * p - 4 * j >= 2 else 0
                nc.scalar.activation(at[:, :, skip:512],
                                     pst[:, :, skip:512],
                                     mybir.ActivationFunctionType.Exp,
                                     scale=0.125)
                for d in (0, 1):
                    a = 2 * p + d - 4 * j
                    if a >= 0:
                        nc.vector.tensor_tensor(
                            at[:, d, 128 * a:128 * a + 128],
                            at[:, d, 128 * a:128 * a + 128],
                            masks[:, 384:512], mybir.AluOpType.mult)
                at_tiles[idx] = (at, f8)

            def emit_A(idx):
                h, j, p, npair = items[idx]
                at, f8 = at_tiles.pop(idx)
                pav = pavs[(h, j)]
                if f8:
                    nc.tensor.matmul(pav[:], v_sb[:, 2 * p:2 * p + 2, h, :],
                                     at[:], start=(p == 0), stop=False,
                                     perf_mode=DR)
                else:
                    for d in (0, 1):
                        i = 2 * p + d
                        a = i - 4 * j
                        lead = 128 * a if a > 0 else 0
                        nc.tensor.matmul(pav[:, lead:512], v_sb[:, i, h, :],
                                         at[:, d, lead:512],
                                         start=(i == 0),
                                         stop=(i == 2 * npair - 1))
                if p == npair - 1:
                    flush_queue.append((pavs.pop((h, j)), h, j))
                    if h == 2:  # last unit of block j
                        make_proj(j)

            for idx in range(len(items) + 1):
                if idx < len(items):
                    emit_S(idx)
                    h, j, p, npair = items[idx]
                    # flush the previous unit early in this unit: its pb
                    # broadcast fills the PE while this unit's exp completes
                    if p == 1 and flush_queue:
                        emit_flush()
                if idx >= 1:
                    emit_A(idx - 1)
                # inject deferred v eagerly; keep 3 proj closures in reserve
                # to feed the PE through the end-of-kernel flush chain
                if fill_queue:
                    if idx >= 1:
                        fill_queue.pop(0)()
                elif proj_queue and len(proj_queue) > (4 if j == 3 else 3):
                    proj_queue.pop(0)()

            # Drain: reserve closures first (ready now -- keep the PE busy
            # while the final flush chain resolves), final flush in between,
            # dummy matmuls bridging the gaps so the drain stays at 2.4 GHz.
            pwarm = psav.tile([VD, 512], F32, tag="psav", name="pwarm2")
            DRAIN["on"] = True
            while flush_queue or proj_queue:
                for _ in range(3):
                    if proj_queue:
                        proj_queue.pop(0)()
                    dummy_mm(1)
                if flush_queue:
                    emit_flush(fast=True)

    nc.compile()
    return nc


def _shard_inputs(x, w_attn, b_attn, lora_a_attn, lora_b_attn, w_proj, b_proj,
                  lora_a_proj, lora_b_proj):
    import ml_dtypes
    bf16 = ml_dtypes.bfloat16
    fp8 = ml_dtypes.float8_e4m3fn
    xdt = fp8 if FP8_QKV else bf16
    pdt = fp8 if FP8_PROJ else bf16
    vdt = fp8 if FP8_AV else bf16
    f32 = np.float32
    x = np.asarray(x, f32)
    w_attn = np.asarray(w_attn, f32)
    b_attn = np.asarray(b_attn, f32)
    lora_a_attn = np.asarray(lora_a_attn, f32)
    lora_b_attn = np.asarray(lora_b_attn, f32)
    w_proj = np.asarray(w_proj, f32)
    b_proj = np.asarray(b_proj, f32)
    lora_a_proj = np.asarray(lora_a_proj, f32)
    lora_b_proj = np.asarray(lora_b_proj, f32)

    def shuf(a, nchunk):
        # (nchunk*128, F) row-major -> (128, nchunk, F) partition-major
        return np.ascontiguousarray(
            a.reshape(nchunk, P, -1).transpose(1, 0, 2))

    laa2 = shuf(lora_a_attn.T * WS, CT).reshape(P, -1).astype(xdt)
    # masks[p, z] = 1.0 if z >= p + 384 else 0.0
    pp, zz = np.meshgrid(np.arange(P), np.arange(896), indexing="ij")
    masks = (zz >= pp + 384).astype(bf16)
    onesr = np.ones((1, T), bf16)
    vpat = np.zeros((P, T1 * HPC, VD - HD), f32)
    vpat[:, :, 0] = 1.0
    vones = np.ascontiguousarray(vpat.reshape(P, -1)).astype(vdt)
    lbp = np.concatenate(
        [(lora_b_proj * LORA_SCALE).T, np.zeros((1, C), f32)], 0) * PS
    in_maps = []
    for core in range(N_CORES):
        b = core // GPB
        heads = [(core % GPB) * HPC + k for k in range(HPC)]
        q_idx = np.concatenate([np.arange(h * HD, (h + 1) * HD) for h in heads])
        k_idx = q_idx + C
        v_idx = q_idx + 2 * C
        qk_idx = np.concatenate([q_idx, k_idx])
        wqk2 = shuf(w_attn[qk_idx].T * WS, CT).reshape(P, -1).astype(xdt)
        wv2 = shuf(w_attn[v_idx].T, CT).reshape(P, -1).astype(bf16)
        lbaqk = np.concatenate(
            [(lora_b_attn[qk_idx] * LORA_SCALE).T, b_attn[qk_idx][None, :]],
            0) * WS
        lbav = np.concatenate(
            [(lora_b_attn[v_idx] * LORA_SCALE).T, b_attn[v_idx][None, :]], 0)
        wp2 = shuf(w_proj[:, q_idx].T * PS, 2).reshape(P, -1).astype(pdt)
        lap2 = shuf(lora_a_proj[:, q_idx].T * PS, 2).reshape(P, -1).astype(pdt)
        lbp_c = lbp.copy()
        if core % GPB == 0:
            lbp_c[R] = b_proj * PS
        in_maps.append({
            "xt": np.ascontiguousarray(x[b].T).astype(xdt),
            "wqk": wqk2, "wv": wv2,
            "laa": laa2, "lbaqk": np.ascontiguousarray(lbaqk).astype(bf16),
            "lbav": np.ascontiguousarray(lbav).astype(bf16),
            "wp": wp2, "lap": lap2,
            "lbp": np.ascontiguousarray(lbp_c).astype(bf16),
            "masks": masks, "onesr": onesr, "vones": vones,
        })
    return in_maps


def kernel(x, w_attn, b_attn, lora_a_attn, lora_b_attn, w_proj, b_proj,
           lora_a_proj, lora_b_proj, n_head):
    global LAST_RESULTS
    assert int(n_head) == H
    if "nc" not in _CACHE:
        _CACHE["nc"] = build()
    nc = _CACHE["nc"]
    in_maps = _shard_inputs(x, w_attn, b_attn, lora_a_attn, lora_b_attn,
                            w_proj, b_proj, lora_a_proj, lora_b_proj)
    res = run_bass_kernel_spmd(
        nc, in_maps, core_ids=list(range(N_CORES)),
        trace=bool(os.environ.get("BASS_KERNEL_TRACE")))
    LAST_RESULTS = res
    out = np.zeros((B, C, T), np.float32)
    for core in range(N_CORES):
        out[core // GPB] += np.asarray(res.results[core]["out"],
                                       dtype=np.float32)
    return np.ascontiguousarray(out.transpose(0, 2, 1))


# revision 44
# speedup vs baseline: 1.1240x; 1.1238x over previous
"""Trainium2 Bass kernel: causal multi-head attention with LoRA (B=2, T=2048,
C=1024, 16 heads, r=16), SPMD across 8 NeuronCores.

Sharding: core = (batch, head-group-of-4). QKV + attention are fully local per
core (weights pre-sliced per head group on host); the output projection is
computed as a partial sum over each core's 256 y-features and reduced on host.

v4 design:
- LoRA is merged on the host (W_eff = W + scale*B@A -- exact for all inputs),
  eliminating the whole low-rank pipeline on-chip (u, up, rank-17 matmuls).
  Biases ride the ACTIVATE bias operand (per-partition for qk / out) and one
  ones-row matmul for v (bias varies along v's free dim).
- All operands bf16 (PSUM accumulation fp32). fp8 was measured at 5.7e-2 rel
  err: for zero-mean dot products the per-element quantization error does not
  average out (signal and error both grow as sqrt(n)).
- qk features f0 + f2(j0,j1) accumulate c-major, streaming behind the x-tile
  DMAs, so the PE is busy through the input-DMA window.
- Phase 4 (attention) is one globally software-pipelined stream of k-tile
  pairs (scores one pair ahead of AV, across unit boundaries) so the PE never
  waits on ScalarE's exp and the HAM clock gate stays at 8/8 (2.4 GHz).
  exp is batched [128,1024] per ACTIVATE; in diagonal tile a the leading
  128a query columns are skipped in scores/exp/AV and the causal mask
  multiply is one fixed 128-wide triangular strip.
- Normalize: D row -> DVE reciprocal_approx_fast -> GPSIMD partition
  broadcast (PE-matmul broadcast for the end-of-kernel flush) -> multiply.
- v for key tiles 6..15 and the j-1 projection interleave into attention
  block j; reserve closures + scratch matmuls keep the clock gate warm
  through the DMA-paced start and the end-of-kernel drain.
- Output partials ship as bf16, summed in f32 on the host.
"""
import os
import sys

sys.path.insert(0, "/opt/trn_rl_repo")

import numpy as np

import concourse.bass as bass  # noqa: F401
import concourse.bacc as bacc
import concourse.tile as tile
import concourse.mybir as mybir
from concourse.bass_utils import run_bass_kernel_spmd

B, T, C = 2, 2048, 1024
H, HD = 16, 64
R = 16
LORA_SCALE = 1.0 / R
N_CORES = 8
GPB = N_CORES // B          # core groups per batch = 4
HPC = H // GPB              # heads per core = 4
CI = HPC * HD               # per-core y features = 256
P = 128
T5 = T // 512               # 4  (512-wide query blocks)
T1 = T // P                 # 16 (128-wide key tiles)
CT = C // P                 # 8  (128-wide c tiles)
FQK = 2 * HPC * HD // P     # 4  (128-wide qk feature tiles: f0,f1=q f2,f3=k)
F32 = mybir.dt.float32
BF16 = mybir.dt.bfloat16
VD = 65                     # v tile cols (64 + ones column for denominators)

LAST_RESULTS = None
_CACHE = {}


def build():
    nc = bacc.Bacc("TRN2", target_bir_lowering=False, debug=False,
                   num_devices=N_CORES)

    xt_d = nc.dram_tensor("xt", [C, T], BF16, kind="ExternalInput").ap()
    wqk_d = nc.dram_tensor("wqk", [P, CT * FQK * P], BF16, kind="ExternalInput").ap()
    wv_d = nc.dram_tensor("wv", [P, CT * CI], BF16, kind="ExternalInput").ap()
    wp_d = nc.dram_tensor("wp", [P, 2 * CT * P], BF16, kind="ExternalInput").ap()
    bqk_d = nc.dram_tensor("bqk", [P, FQK], F32, kind="ExternalInput").ap()
    bv_d = nc.dram_tensor("bv", [1, CI], BF16, kind="ExternalInput").ap()
    bp_d = nc.dram_tensor("bp", [P, CT], F32, kind="ExternalInput").ap()
    masks_d = nc.dram_tensor("masks", [P, 896], BF16, kind="ExternalInput").ap()
    onesr_d = nc.dram_tensor("onesr", [1, P], BF16, kind="ExternalInput").ap()
    vones_d = nc.dram_tensor("vones", [P, T1 * HPC], BF16,
                             kind="ExternalInput").ap()
    out_d = nc.dram_tensor("out", [C, T], BF16, kind="ExternalOutput").ap()

    with tile.TileContext(nc) as tc:
        with (
            tc.tile_pool(name="const", bufs=1) as cp,
            tc.tile_pool(name="work", bufs=2) as wk,
            tc.tile_pool(name="att", bufs=3) as ap_,
            tc.tile_pool(name="ps", bufs=2, space="PSUM") as ps,
            tc.tile_pool(name="pss", bufs=2, space="PSUM") as pss,
            tc.tile_pool(name="psav", bufs=2, space="PSUM") as psav,
        ):
            # ---- resident SBUF tensors -------------------------------------
            xt_sb = cp.tile([P, CT, T], BF16)            # x^T
            wqk_sb = cp.tile([P, CT, FQK, P], BF16)      # W_qk_eff^T
            wv_sb = cp.tile([P, CT, CI], BF16)           # W_v_eff^T
            wp_sb = cp.tile([P, 2, CT, P], BF16)         # W_proj_eff^T slice
            bqk_sb = cp.tile([P, FQK], F32)
            bv_sb = cp.tile([1, CI], BF16)
            bp_sb = cp.tile([P, CT], F32)
            qk_sb = cp.tile([P, FQK, T], BF16)           # q,k feat-major
            v_sb = cp.tile([P, T1, HPC, VD], BF16)       # v natural + ones
            yt_sb = cp.tile([P, 2, T], BF16)             # y^T (ci-major)
            masks = cp.tile([P, 896], BF16)              # causal masks
            onesb = cp.tile([1, P], BF16)                # ones row

            # ---- input DMAs (few, ordered; sync issues ~0.65us each) -------
            for c in range(2):
                nc.sync.dma_start(out=xt_sb[:, c, :], in_=xt_d[c * P:(c + 1) * P, :])
            nc.sync.dma_start(out=wqk_sb[:, 0:4, :, :],
                              in_=wqk_d[:, 0:4 * FQK * P])
            nc.sync.dma_start(out=wqk_sb[:, 4:8, :, :],
                              in_=wqk_d[:, 4 * FQK * P:])
            nc.sync.dma_start(out=bqk_sb[:], in_=bqk_d[:])
            for c in range(2, CT):
                nc.sync.dma_start(out=xt_sb[:, c, :], in_=xt_d[c * P:(c + 1) * P, :])
            nc.sync.dma_start(out=wv_sb[:], in_=wv_d[:])
            nc.sync.dma_start(out=bv_sb[:], in_=bv_d[:])
            nc.sync.dma_start(out=onesb[:], in_=onesr_d[:])
            nc.sync.dma_start(out=masks[:], in_=masks_d[:])
            nc.sync.dma_start(out=v_sb[:, :, :, HD:VD], in_=vones_d[:])
            nc.sync.dma_start(out=wp_sb[:], in_=wp_d[:])
            nc.sync.dma_start(out=bp_sb[:], in_=bp_d[:])

            # ---- PE pre-warm -----------------------------------------------
            # The HAM clock gate needs ~3.4us of sustained PE activity to
            # raise the PE clock from 1.2 to 2.4 GHz; scratch matmuls during
            # the DMA window make the first real matmuls run at full clock.
            scr = cp.tile([P, 640], BF16)
            nc.vector.memset(scr[:], 0)
            pwarm = psav.tile([VD, 512], F32, tag="psav", name="pwarm")

            def dummy_mm(n):
                for _ in range(n):
                    nc.tensor.matmul(pwarm[:], scr[:, 0:VD],
                                     scr[:, P:640], start=True, stop=True)

            dummy_mm(16)

            # ---- phase 2 qk, f0 + f2(j0,j1) streamed behind the x DMAs -----
            # Six matmuls per landing x tile ~= the DMA arrival rate, so the
            # PE streams through the input window instead of idling.
            pqf0 = [pss.tile([P, 2, 512], F32, tag="pss", name=f"pqf0_{k}")
                    for k in range(2)]
            pqf2 = [ps.tile([P, 512], F32, tag="ps", name=f"pqf2_{k}")
                    for k in range(2)]
            for c in range(CT):
                for j in range(T5):
                    nc.tensor.matmul(pqf0[j // 2][:, j % 2, :],
                                     wqk_sb[:, c, 0, :],
                                     xt_sb[:, c, j * 512:(j + 1) * 512],
                                     start=(c == 0), stop=(c == CT - 1))
                for j in range(2):
                    nc.tensor.matmul(pqf2[j][:], wqk_sb[:, c, 2, :],
                                     xt_sb[:, c, j * 512:(j + 1) * 512],
                                     start=(c == 0), stop=(c == CT - 1))
            for j in range(T5):
                nc.scalar.activation(qk_sb[:, 0, j * 512:(j + 1) * 512],
                                     pqf0[j // 2][:, j % 2, :],
                                     mybir.ActivationFunctionType.Identity,
                                     bias=bqk_sb[:, 0:1])
            for j in range(2):
                nc.scalar.activation(qk_sb[:, 2, j * 512:(j + 1) * 512],
                                     pqf2[j][:],
                                     mybir.ActivationFunctionType.Identity,
                                     bias=bqk_sb[:, 2:3])

            # ---- phase 2 remainder: f2(j2,j3), f1, f3 ----------------------
            for f, jr in ((2, (2, 3)), (1, (0, 1, 2, 3)), (3, (0, 1, 2, 3))):
                for j in jr:
                    pq = ps.tile([P, 512], F32, tag="ps", name=f"pq{f}_{j}")
                    for c in range(CT):
                        nc.tensor.matmul(pq[:], wqk_sb[:, c, f, :],
                                         xt_sb[:, c, j * 512:(j + 1) * 512],
                                         start=(c == 0), stop=(c == CT - 1))
                    nc.scalar.activation(
                        qk_sb[:, f, j * 512:(j + 1) * 512], pq[:],
                        mybir.ActivationFunctionType.Identity,
                        bias=bqk_sb[:, f:f + 1])

            # ---- phase 3: V natural = x @ Wv_eff^T + ones ⊗ bv -------------
            # Key tiles 6..15 are only consumed by later attention blocks, so
            # their v computation defers into blocks 0/1 as PE filler.
            _pv = {}

            def emit_v(i, part=None):
                if part in (None, 0):
                    _pv[i] = ps.tile([P, HPC, HD], F32, tag="ps",
                                     name=f"pv{i}")
                pv = _pv[i]
                cr = {None: range(CT), 0: range(3), 1: range(3, 6),
                      2: range(6, CT)}[part]
                for c in cr:
                    nc.tensor.matmul(pv[:], xt_sb[:, c, i * P:(i + 1) * P],
                                     wv_sb[:, c, :],
                                     start=(c == 0), stop=False)
                if part in (None, 2):
                    nc.tensor.matmul(pv[:], onesb[:], bv_sb[:],
                                     start=False, stop=True)
                    nc.scalar.copy(v_sb[:, i, :, 0:HD], pv[:])
                    del _pv[i]

            for i in range(6):
                emit_v(i)

            # ---- phase 4 + interleaved projection --------------------------
            DRAIN = {"on": False}
            flush_queue = []   # units awaiting the normalize chain
            fill_queue = [(lambda i=i, pt=pt: emit_v(i, pt))
                          for i in range(6, T1) for pt in range(3)]
            proj_queue = []    # closures: one PE-group of proj work each

            def emit_flush(fast=False):
                pav, h, j = flush_queue.pop(0)
                # D row (PSUM) -> SBUF, reciprocal, broadcast to 64
                # partitions, then y^T = yu^T * (1/D). Broadcast runs on
                # GPSIMD (off the PE) except for the end-of-kernel flush,
                # where the PE-matmul broadcast chain is ~1.2us shorter and
                # the PE is idle anyway.
                rsb = wk.tile([HD, 512], F32, tag="rsb", name=f"rsb{h}_{j}")
                if fast:
                    bsb = wk.tile([1, 512], BF16, tag="bsbf",
                                  name=f"bsbf{h}_{j}")
                    nc.vector.tensor_scalar_add(bsb[:], pav[HD:HD + 1, :], 0.0)
                    pb = ps.tile([HD, 512], F32, tag="ps", name=f"pb{h}_{j}")
                    nc.tensor.matmul(pb[:], onesb[:, 0:HD], bsb[:],
                                     start=True, stop=True)
                    nc.vector.reciprocal_approx_fast(rsb[:], pb[:])
                else:
                    bsb = wk.tile([1, 512], F32, tag="bsb", name=f"bsb{h}_{j}")
                    nc.vector.tensor_scalar_add(bsb[:], pav[HD:HD + 1, :], 0.0)
                    rrow = wk.tile([1, 512], F32, tag="rrow",
                                   name=f"rr{h}_{j}")
                    nc.vector.reciprocal_approx_fast(rrow[:], bsb[:])
                    nc.gpsimd.partition_broadcast(rsb[:], rrow[:])
                if h % 2 == 0:
                    nc.vector.tensor_tensor(
                        yt_sb[0:HD, h // 2, j * 512:(j + 1) * 512],
                        pav[0:HD, :], rsb[:], mybir.AluOpType.mult)
                else:
                    tsb = wk.tile([HD, 512], BF16, tag="tsb",
                                  name=f"tsb{h}_{j}")
                    nc.vector.tensor_tensor(tsb[:], pav[0:HD, :], rsb[:],
                                            mybir.AluOpType.mult)
                    nc.sync.dma_start(
                        out=yt_sb[HD:P, h // 2, j * 512:(j + 1) * 512],
                        in_=tsb[:])

            def make_proj(j):
                for co in range(CT):
                    def p6(co=co):
                        po = ps.tile([P, 512], F32, tag="ps",
                                     name=f"po{j}_{co}")
                        for ci in range(2):
                            nc.tensor.matmul(
                                po[:], wp_sb[:, ci, co, :],
                                yt_sb[:, ci, j * 512:(j + 1) * 512],
                                start=(ci == 0), stop=(ci == 1))
                        oq = wk.tile([P, 512], BF16, tag="oq",
                                     name=f"oq{j}_{co}")
                        nc.vector.tensor_scalar_add(oq[:], po[:],
                                                    bp_sb[:, co:co + 1])
                        nc.sync.dma_start(
                            out=out_d[co * P:(co + 1) * P,
                                      j * 512:(j + 1) * 512],
                            in_=oq[:])
                    proj_queue.append(p6)

            # Flatten all (head, block) units into one stream of k-tile pairs
            # and software-pipeline ACROSS unit boundaries. Odd heads first
            # per block: their normalize chain ends in an SBUF DMA (partition
            # shift), so the block-final flush is a direct DVE write.
            items = []
            for j in range(T5):
                for h in (1, 3, 0, 2):
                    npair = 2 * (j + 1)
                    for p in range(npair):
                        items.append((h, j, p, npair))

            pavs = {}
            at_tiles = {}

            def emit_S(idx):
                h, j, p, npair = items[idx]
                if p == 0:
                    pavs[(h, j)] = psav.tile([VD, 512], F32, tag="psav",
                                             name=f"pav{h}_{j}")
                # Diagonal k-tile pairs align as (a=0,1) or (a=2,3), where
                # a = i - 4j. In diagonal tile a, query columns < 128a are
                # entirely above the causal diagonal: skip them in the score
                # matmul, the exp, and the AV matmul. Only the 128-wide
                # partial strip [128a, 128a+128) needs the mask multiply,
                # and it is the same triangular block masks[:, 384:512].
                pst = pss.tile([P, 2, 512], F32, tag="pss",
                               name=f"pst{h}_{j}_{p}")
                fq = h // 2
                pqb = (h % 2) * HD
                for d in (0, 1):
                    i = 2 * p + d
                    a = i - 4 * j
                    lead = 128 * a if a > 0 else 0
                    qt = qk_sb[pqb:pqb + HD, fq,
                               j * 512 + lead:(j + 1) * 512]
                    kt = qk_sb[pqb:pqb + HD, 2 + fq, i * P:(i + 1) * P]
                    nc.tensor.matmul(pst[:, d, lead:512], kt, qt,
                                     start=True, stop=True)
                at = ap_.tile([P, 2, 512], BF16, tag="att",
                              name=f"at{h}_{j}_{p}")
                skip = 256 if 2 * p - 4 * j >= 2 else 0
                nc.scalar.activation(at[:, :, skip:512],
                                     pst[:, :, skip:512],
                                     mybir.ActivationFunctionType.Exp,
                                     scale=0.125)
                for d in (0, 1):
                    a = 2 * p + d - 4 * j
                    if a >= 0:
                        nc.vector.tensor_tensor(
                            at[:, d, 128 * a:128 * a + 128],
                            at[:, d, 128 * a:128 * a + 128],
                            masks[:, 384:512], mybir.AluOpType.mult)
                at_tiles[idx] = at

            def emit_A(idx):
                h, j, p, npair = items[idx]
                at = at_tiles.pop(idx)
                pav = pavs[(h, j)]
                for d in (0, 1):
                    i = 2 * p + d
                    a = i - 4 * j
                    lead = 128 * a if a > 0 else 0
                    nc.tensor.matmul(pav[:, lead:512], v_sb[:, i, h, :],
                                     at[:, d, lead:512],
                                     start=(i == 0),
                                     stop=(i == 2 * npair - 1))
                if p == npair - 1:
                    flush_queue.append((pavs.pop((h, j)), h, j))
                    if h == 2:  # last unit of block j
                        make_proj(j)

            for idx in range(len(items) + 1):
                if idx < len(items):
                    emit_S(idx)
                    h, j, p, npair = items[idx]
                    # flush the previous unit early in this unit: its chain
                    # fills engines while this unit's exp completes
                    if p == 1 and flush_queue:
                        emit_flush()
                if idx >= 1:
                    emit_A(idx - 1)
                # inject deferred v eagerly; keep proj closures in reserve to
                # feed the PE through the end-of-kernel flush chain
                if fill_queue:
                    if idx >= 1:
                        fill_queue.pop(0)()
                elif proj_queue and len(proj_queue) > (4 if j == 3 else 3):
                    proj_queue.pop(0)()

            # Drain: reserve closures first (ready now -- keep the PE busy
            # while the final flush chain resolves), final flush in between,
            # scratch matmuls bridging gaps so the drain stays at 2.4 GHz.
            pwarm = psav.tile([VD, 512], F32, tag="psav", name="pwarm2")
            DRAIN["on"] = True
            while flush_queue or proj_queue:
                for _ in range(3):
                    if proj_queue:
                        proj_queue.pop(0)()
                    dummy_mm(1)
                if flush_queue:
                    emit_flush(fast=True)

    nc.compile()
    return nc


def _shard_inputs(x, w_attn, b_attn, lora_a_attn, lora_b_attn, w_proj, b_proj,
                  lora_a_proj, lora_b_proj):
    import ml_dtypes
    bf16 = ml_dtypes.bfloat16
    f32 = np.float32
    x = np.asarray(x, f32)
    b_attn = np.asarray(b_attn, f32)
    b_proj = np.asarray(b_proj, f32)
    # Merge LoRA into the dense weights (exact: y = x@(W + s*B@A)^T + b).
    w_attn = np.asarray(w_attn, f32) + LORA_SCALE * (
        np.asarray(lora_b_attn, f32) @ np.asarray(lora_a_attn, f32))
    w_proj = np.asarray(w_proj, f32) + LORA_SCALE * (
        np.asarray(lora_b_proj, f32) @ np.asarray(lora_a_proj, f32))

    def shuf(a, nchunk):
        # (nchunk*128, F) row-major -> (128, nchunk, F) partition-major
        return np.ascontiguousarray(
            a.reshape(nchunk, P, -1).transpose(1, 0, 2))

    # masks[p, z] = 1.0 if z >= p + 384 else 0.0
    pp, zz = np.meshgrid(np.arange(P), np.arange(896), indexing="ij")
    masks = (zz >= pp + 384).astype(bf16)
    onesr = np.ones((1, P), bf16)
    vones = np.ones((P, T1 * HPC), bf16)
    in_maps = []
    for core in range(N_CORES):
        b = core // GPB
        heads = [(core % GPB) * HPC + k for k in range(HPC)]
        q_idx = np.concatenate([np.arange(h * HD, (h + 1) * HD) for h in heads])
        k_idx = q_idx + C
        v_idx = q_idx + 2 * C
        qk_idx = np.concatenate([q_idx, k_idx])
        wqk2 = shuf(w_attn[qk_idx].T, CT).reshape(P, -1).astype(bf16)
        wv2 = shuf(w_attn[v_idx].T, CT).reshape(P, -1).astype(bf16)
        wp2 = shuf(w_proj[:, q_idx].T, 2).reshape(P, -1).astype(bf16)
        bqk = np.ascontiguousarray(
            b_attn[qk_idx].reshape(FQK, P).T).astype(f32)
        bv = np.ascontiguousarray(b_attn[v_idx][None, :]).astype(bf16)
        bpf = b_proj if core % GPB == 0 else np.zeros((C,), f32)
        bp = np.ascontiguousarray(bpf.reshape(CT, P).T).astype(f32)
        in_maps.append({
            "xt": np.ascontiguousarray(x[b].T).astype(bf16),
            "wqk": wqk2, "wv": wv2, "wp": wp2,
            "bqk": bqk, "bv": bv, "bp": bp,
            "masks": masks, "onesr": onesr, "vones": vones,
        })
    return in_maps


def kernel(x, w_attn, b_attn, lora_a_attn, lora_b_attn, w_proj, b_proj,
           lora_a_proj, lora_b_proj, n_head):
    global LAST_RESULTS
    assert int(n_head) == H
    if "nc" not in _CACHE:
        _CACHE["nc"] = build()
    nc = _CACHE["nc"]
    in_maps = _shard_inputs(x, w_attn, b_attn, lora_a_attn, lora_b_attn,
                            w_proj, b_proj, lora_a_proj, lora_b_proj)
    res = run_bass_kernel_spmd(
        nc, in_maps, core_ids=list(range(N_CORES)),
        trace=bool(os.environ.get("BASS_KERNEL_TRACE")))
    LAST_RESULTS = res
    out = np.zeros((B, C, T), np.float32)
    for core in range(N_CORES):
        out[core // GPB] += np.asarray(res.results[core]["out"],
                                       dtype=np.float32)
    return np.ascontiguousarray(out.transpose(0, 2, 1))
